# revision 2
# baseline (speedup 1.0000x reference)
"""MeshCNN-style MeshConv kernel for Trainium2 (8 NeuronCores, Bass/Tile).

Problem: x (4, 16, 500000, 5) f32, W (16, 16, 1, 5) f32, b (16,) f32.
  g = [x0, x1+x3, x2+x4, |x1-x3|, |x2-x4|] stacked on a new axis (h, size 5)
  y = conv2d(g, W, kernel (1,5), VALID) + b    -> (4, 16, 5, 499996) f32

Strategy (memory-bound target):
  - Host precomputes the 5 combined g planes in f32, casts to bf16
    (elementwise combine is layout/precision-trivial; keeps the device
    kernel pure load->matmul->store).
  - Shard the F (face) axis across the 8 cores (62504 output faces each).
  - Banded-weight matmul: partition dim packs (ci, j) with j = face
    position within a chunk of 8 faces.  One dense 128x128 weight whose
    (ci*8+j, co*8+j') entry is W[co,ci,j-j'] computes all 5 conv taps
    for 8 output faces at once; a second accumulating matmul with the
    next face-chunk (taps crossing the chunk boundary) completes the
    band.  5 cycles/face vs 25 for a block-diagonal layout.
  - Columns are (chunk, n) with n fastest, so the second matmul's rhs
    is the same buffer at a +4 column offset: both matmuls are
    contiguous 2D slices.  Per tile of 128 chunks: 10 matmuls of 512
    cols, 5 psum banks (one per h), DVE/ACT evictions with fused bias,
    one 660KB DMA in (sync ring) and one 655KB DMA out (scalar ring).
"""

import os
import sys

import numpy as np

if "/opt/trn_rl_repo" not in sys.path:
    sys.path.insert(0, "/opt/trn_rl_repo")

import ml_dtypes

N, CI, CO, F, K = 4, 16, 16, 500000, 5
FO_TOTAL = F - (K - 1)            # 499996 valid output faces
NCORES = 8
J = 8                             # faces per chunk (partition-packed)
CH_CORE = 7813                    # output chunks per core (62504 faces)
FO_CORE = CH_CORE * J             # 62504
C_TILE = 128                      # chunks per tile (=> 512-col matmuls)
F_PAD = NCORES * FO_CORE + J      # padded global face count for g (500040)

_NC_CACHE = {}


def _tiles_for(nchunks=CH_CORE, tile=C_TILE):
    tiles = []
    c0 = 0
    while c0 < nchunks:
        tiles.append((c0, min(tile, nchunks - c0)))
        c0 += tile
    return tiles


def _col_layout():
    """Per-tile x/y column offsets (in elements) into the flat dram rows."""
    tiles = _tiles_for()
    xoffs, yoffs = [], []
    xo = yo = 0
    for _, c in tiles:
        xoffs.append(xo)
        yoffs.append(yo)
        xo += K * (c + 1) * N      # 5 h-planes, (c+1) chunks (halo), n fast
        yo += K * c * N
    return tiles, xoffs, yoffs, xo, yo


def build_nc():
    """Build the (SPMD, per-core) Bass kernel. Same NEFF for every core."""
    import concourse.mybir as mybir
    import concourse.tile as tile
    from concourse import bacc

    dt = mybir.dt
    nc = bacc.Bacc("TRN2", target_bir_lowering=False, debug=False,
                   enable_asserts=False)

    tiles, xoffs, yoffs, XCOLS, YCOLS = _col_layout()

    x_d = nc.dram_tensor("x", [128, XCOLS], dt.bfloat16, kind="ExternalInput")
    w_d = nc.dram_tensor("w", [128, 2 * 128], dt.bfloat16,
                         kind="ExternalInput")
    b_d = nc.dram_tensor("b", [128, 1], dt.float32, kind="ExternalInput")
    y_d = nc.dram_tensor("y", [128, YCOLS], dt.bfloat16, kind="ExternalOutput")

    x_ap = x_d.ap()
    y_ap = y_d.ap()

    with tile.TileContext(nc) as tc:
        with (
            tc.tile_pool(name="const", bufs=1) as cpool,
            tc.tile_pool(name="xp", bufs=4) as xp,
            tc.tile_pool(name="yp", bufs=4) as yp,
            tc.tile_pool(name="ps", bufs=8, space="PSUM") as pp,
        ):
            # constants on the GpSimd (SWDGE) queue: doesn't delay the
            # first x-tile DMAs on the sync HWDGE ring
            Wt = cpool.tile([128, 2 * 128], dt.bfloat16)
            nc.gpsimd.dma_start(Wt[:], w_d.ap())
            bt = cpool.tile([128, 1], dt.float32)
            nc.gpsimd.dma_start(bt[:], b_d.ap())

            for ti, (c0, c) in enumerate(tiles):
                xlen = K * (c + 1) * N
                X = xp.tile([128, xlen], dt.bfloat16, tag="X")
                nc.sync.dma_start(X[:], x_ap[:, xoffs[ti]:xoffs[ti] + xlen])

                cols = c * N                     # matmul free size (<=512)
                Y = yp.tile([128, K * cols], dt.bfloat16, tag="Y")
                for h in range(K):
                    ps = pp.tile([128, cols], dt.float32, tag="ps",
                                 name=f"ps{h}")
                    hoff = h * (c + 1) * N
                    nc.tensor.matmul(ps[:], Wt[:, 0:128],
                                     X[:, hoff:hoff + cols],
                                     start=True, stop=False)
                    nc.tensor.matmul(ps[:], Wt[:, 128:256],
                                     X[:, hoff + N:hoff + N + cols],
                                     start=False, stop=True)
                    # psum->sbuf eviction with fused bias, split DVE/ACT
                    if h % 2 == 0:
                        nc.vector.tensor_scalar_add(
                            Y[:, h * cols:(h + 1) * cols], ps[:], bt[:])
                    else:
                        nc.scalar.activation(
                            Y[:, h * cols:(h + 1) * cols], ps[:],
                            mybir.ActivationFunctionType.Identity,
                            bias=bt[:])
                nc.scalar.dma_start(
                    y_ap[:, yoffs[ti]:yoffs[ti] + K * cols], Y[:])
    nc.compile()
    return nc


def _get_nc():
    if "nc" not in _NC_CACHE:
        _NC_CACHE["nc"] = build_nc()
    return _NC_CACHE["nc"]


def _make_weight_inputs(W, b):
    """Banded 128x[2*128] bf16 weights + per-partition bias (128,1) f32.

    lhsT1[ci*8+j, co*8+j'] = W[co,ci,j-j']   for 0 <= j-j'  <= 4
    lhsT2[ci*8+j, co*8+j'] = W[co,ci,j+8-j'] for 0 <= j+8-j' <= 4
    """
    W = np.asarray(W, dtype=np.float32).reshape(CO, CI, K)
    LT = np.zeros((2, 128, 128), dtype=np.float32)
    for jj in range(J):
        for jp in range(J):
            k1 = jj - jp
            if 0 <= k1 < K:
                LT[0, jj::J, jp::J] = W[:, :, k1].T   # rows ci*8+jj
            k2 = jj + J - jp
            if 0 <= k2 < K:
                LT[1, jj::J, jp::J] = W[:, :, k2].T
    # rows ci*8+jj: LT[0][ci*8+jj, co*8+jp]; the ::J slicing above gives
    # [jj::J, jp::J] -> index [ci, co] which is W[:, :, k].T == [ci, co]. OK
    LTb = np.concatenate([LT[0], LT[1]], axis=1).astype(ml_dtypes.bfloat16)
    bias = np.repeat(np.asarray(b, dtype=np.float32).reshape(CO), J)
    return np.ascontiguousarray(LTb), np.ascontiguousarray(
        bias.reshape(128, 1))


def _combine_g(x):
    """Host combine in f32 -> bf16 planes, padded to F_PAD faces."""
    x = np.asarray(x, dtype=np.float32)
    g = np.zeros((N, CI, K, F_PAD), dtype=ml_dtypes.bfloat16)
    g[:, :, 0, :F] = x[:, :, :, 0]
    g[:, :, 1, :F] = x[:, :, :, 1] + x[:, :, :, 3]
    g[:, :, 2, :F] = x[:, :, :, 2] + x[:, :, :, 4]
    g[:, :, 3, :F] = np.abs(x[:, :, :, 1] - x[:, :, :, 3])
    g[:, :, 4, :F] = np.abs(x[:, :, :, 2] - x[:, :, :, 4])
    return g


def _shard_x(g):
    """Per-core (128, XCOLS) bf16 shards in banded (ci,j) layout."""
    tiles, xoffs, _, XCOLS, _ = _col_layout()
    shards = []
    for core in range(NCORES):
        s = core * FO_CORE
        gc = g[:, :, :, s:s + (CH_CORE + 1) * J]       # (n, ci, h, faces)
        gch = np.ascontiguousarray(gc).reshape(N, CI, K, CH_CORE + 1, J)
        xc = np.empty((128, XCOLS), dtype=ml_dtypes.bfloat16)
        for ti, (c0, c) in enumerate(tiles):
            blk = gch[:, :, :, c0:c0 + c + 1, :]       # (n,ci,h,c+1,j)
            blk = blk.transpose(1, 4, 2, 3, 0)         # (ci,j,h,c+1,n)
            xc[:, xoffs[ti]:xoffs[ti] + K * (c + 1) * N] = \
                blk.reshape(128, K * (c + 1) * N)
        shards.append(xc)
    return shards


def _assemble_y(ys):
    tiles, _, yoffs, _, YCOLS = _col_layout()
    y = np.empty((N, CO, K, NCORES * FO_CORE), dtype=np.float32)
    for core in range(NCORES):
        yc = ys[core]                                  # (128, YCOLS) bf16
        s = core * FO_CORE
        for ti, (c0, c) in enumerate(tiles):
            blk = yc[:, yoffs[ti]:yoffs[ti] + K * c * N].astype(np.float32)
            blk = blk.reshape(CO, J, K, c, N).transpose(4, 0, 2, 3, 1)
            y[:, :, :, s + c0 * J: s + (c0 + c) * J] = \
                blk.reshape(N, CO, K, c * J)
    return y[:, :, :, :FO_TOTAL]


LAST_RESULTS = None


def kernel(x, W, b):
    global LAST_RESULTS
    from concourse.bass_utils import run_bass_kernel_spmd

    g = _combine_g(x)
    LTb, bias = _make_weight_inputs(W, b)
    shards = _shard_x(g)
    in_maps = [{"x": shards[c], "w": LTb, "b": bias} for c in range(NCORES)]

    nc = _get_nc()
    trace = bool(int(os.environ.get("KERNEL_TRACE", "0")))
    res = run_bass_kernel_spmd(nc, in_maps, core_ids=list(range(NCORES)),
                               trace=trace)
    LAST_RESULTS = res
    return _assemble_y([r["y"] for r in res.results])
